# revision 1
# baseline (speedup 1.0000x reference)
"""Trainium2 Bass kernel for causal multi-head attention (B=2, L=2048, D=2048,
H=16 heads, DH=128), sharded over 8 NeuronCores.

Sharding: core c handles batch b=c//4 and head-group g=c%4 (4 heads = 512
features). The only cross-core communication is a per-head-chunk fp16
AllGather of attention outputs within each 4-core batch group.

Precision scheme (fp16 datapath, fp32 PSUM accumulation):
- The softmax temperature here is sqrt(128) (the reference multiplies scores
  by sqrt(d_head)), so absolute score errors are amplified ~11x before exp;
  bf16/tf32-level projections would give percent-level output error.
- q/k path runs in compensated precision (~22 effective bits):
  * Q/K projections: fp16 hi*hi main term + the two cross terms (lo*hi,
    hi*lo) in fp8e4m3 with DoubleRow perf mode (256-deep contraction at half
    cycle cost), accumulated in a second PSUM at scale 2^17 and folded in
    during evacuation.
  * qh/kh are re-split on device into fp16 hi+lo (Pool engine), and the
    scores S = qh.kh use 3 fp16 matmuls (hi*hi + hi*lo + lo*hi).
- V path, P = softmax(S), attention output, and the final Wo projection run
  in plain fp16 (errors ~2-4e-4, no softmax amplification).

Per core:
  1. Q/K/V projections; QT/KT in (head-dim, seq) hi+lo layout, V in
     (seq block, feature) layout. Moving panels are 512 wide: fewer, wider
     matmuls amortize LDWEIGHTS (measured ~2x on hardware vs 256-wide).
  2. Causal attention per head over 128-row q blocks, 512-wide score chunks:
     3-term S matmul into PSUM; causal mask applied on the PE itself via an
     extra accumulate-matmul (identity.T @ mask_const) on the diagonal block;
     per-chunk row-max + exp straight out of PSUM (ACT, fused scale/bias,
     row-sum accumulation); chunk-local maxima reconciled afterwards with
     per-chunk correction factors c_jc = exp(scale*(m_jc - m)) folded into
     one P *= c_jc/l pass; P^T via PE transposes batched 4-per-PSUM-bank so
     one DVE copy serves 4 blocks; O^T = V.T @ P^T accumulated per block.
  3. AllGather O^T over the 4-core batch group (fp16, pipelined per head).
  4. out[:, g-slice] = attn_full @ Wo.T[:, g-slice] + bo, accumulated
     head-chunk-major so early gathers start the final matmuls.

Host side only reshapes/transposes/splits inputs (layout preparation) and
concatenates the 8 output slices; all FLOPs run on device.
"""
import sys

sys.path.insert(0, "/opt/trn_rl_repo")

import numpy as np
import ml_dtypes

B, L, D, H = 2, 2048, 2048, 16
DH = D // H          # 128
G = 4                # head-groups (tensor-parallel degree per batch)
HPG = H // G         # heads per group = 4
FPG = HPG * DH       # features per group = 512
P = 128
SCALE = float(DH) ** 0.5
NEG = -1.0e5         # causal mask additive constant (pre-scale)

_COMPILED = None


def _build(variant="main"):
    import concourse.bacc as bacc
    import concourse.tile as tile
    from concourse import mybir
    from contextlib import ExitStack

    F32 = mybir.dt.float32
    F16 = mybir.dt.float16
    AX = mybir.AxisListType
    OP = mybir.AluOpType
    ACTF = mybir.ActivationFunctionType

    nc = bacc.Bacc("TRN2", target_bir_lowering=False, debug=False, num_devices=8)

    # ---- DRAM I/O ----
    F8 = None  # set below
    xqh = nc.dram_tensor("xqh", [D, L], F16, kind="ExternalInput")
    xkh = nc.dram_tensor("xkh", [D, L], F16, kind="ExternalInput")
    xvh = nc.dram_tensor("xvh", [D, L], F16, kind="ExternalInput")
    wqh = nc.dram_tensor("wqh", [D, FPG], F16, kind="ExternalInput")
    wkh = nc.dram_tensor("wkh", [D, FPG], F16, kind="ExternalInput")
    wvh = nc.dram_tensor("wvh", [D, FPG], F16, kind="ExternalInput")
    F8 = mybir.dt.float8e4
    # fp8 correction operands (hi at scale 1/2^5, lo at 2^12/2^17)
    xq8h = nc.dram_tensor("xq8h", [D, L], F8, kind="ExternalInput")
    xq8l = nc.dram_tensor("xq8l", [D, L], F8, kind="ExternalInput")
    xk8h = nc.dram_tensor("xk8h", [D, L], F8, kind="ExternalInput")
    xk8l = nc.dram_tensor("xk8l", [D, L], F8, kind="ExternalInput")
    wq8h = nc.dram_tensor("wq8h", [D, FPG], F8, kind="ExternalInput")
    wq8l = nc.dram_tensor("wq8l", [D, FPG], F8, kind="ExternalInput")
    wk8h = nc.dram_tensor("wk8h", [D, FPG], F8, kind="ExternalInput")
    wk8l = nc.dram_tensor("wk8l", [D, FPG], F8, kind="ExternalInput")
    woT = nc.dram_tensor("woT", [D, FPG], F16, kind="ExternalInput")
    bq = nc.dram_tensor("bq", [FPG, 1], F32, kind="ExternalInput")
    bk = nc.dram_tensor("bk", [FPG, 1], F32, kind="ExternalInput")
    bvb = nc.dram_tensor("bvb", [P, FPG], F32, kind="ExternalInput")
    bob = nc.dram_tensor("bob", [P, FPG], F32, kind="ExternalInput")
    maskh = nc.dram_tensor("maskh", [P, P], F16, kind="ExternalInput")
    identd = nc.dram_tensor("identd", [P, P], F16, kind="ExternalInput")
    out = nc.dram_tensor("out", [L, FPG], F32, kind="ExternalOutput")
    if variant == "timing":
        chain = nc.dram_tensor("chain", [1, 8], F32, kind="ExternalInput")
        dummy = nc.dram_tensor("chaino", [1, 8], F32, kind="ExternalOutput")

    KC = D // P          # 16 contraction chunks
    IB = L // P          # 16 seq blocks of 128
    IPANEL = 512         # projection moving-dim panel
    NPAN = L // IPANEL   # 8

    def drr(t):
        return t.rearrange("(kc p) f -> p kc f", p=P)

    def drr8(t):
        # DoubleRow pack: tile[p, kp, r, :] = row kp*256 + r*128 + p
        return t.rearrange("(kp r p) f -> p kp r f", r=2, p=P)

    KP = D // 256  # 8 DoubleRow contraction chunks

    with tile.TileContext(nc) as tc:
        with ExitStack() as ctx:
            consts = ctx.enter_context(tc.tile_pool(name="consts", bufs=1))

            maskh_t = consts.tile([P, P], F16)
            nc.sync.dma_start(maskh_t[:], maskh[:])
            id_t = consts.tile([P, P], F16)
            nc.sync.dma_start(id_t[:], identd[:])
            bq_t = consts.tile([P, HPG], F32)
            nc.sync.dma_start(bq_t[:], bq.rearrange("(c p) o -> p (c o)", p=P))
            bk_t = consts.tile([P, HPG], F32)
            nc.sync.dma_start(bk_t[:], bk.rearrange("(c p) o -> p (c o)", p=P))
            bvb_t = consts.tile([P, FPG], F32)
            nc.sync.dma_start(bvb_t[:], bvb[:])
            bob_t = consts.tile([P, FPG], F32)
            nc.sync.dma_start(bob_t[:], bob[:])
            if variant == "timing":
                ch_t = consts.tile([1, 8], F32)
                nc.sync.dma_start(ch_t[:], chain[:])
                nc.sync.dma_start(dummy[:], ch_t[:])

            NREP = {"x4": 4, "x2": 2, "x2nc": 2}.get(variant, 1)
            for _rep in range(NREP):
                ag_outs = []
                with tc.tile_pool(name="qkv", bufs=1) as qkv:
                    qth = qkv.tile([P, HPG, L], F16)   # (d, head, seq) hi
                    qtl = qkv.tile([P, HPG, L], F16)   # lo
                    kth = qkv.tile([P, HPG, L], F16)
                    ktl = qkv.tile([P, HPG, L], F16)
                    vt = qkv.tile([P, IB, FPG], F16)   # (seq%128, seq block, feat)

                    # ---- phase 1: projections ----
                    with tc.tile_pool(name="wpool", bufs=2) as wpool, \
                         tc.tile_pool(name="xpool", bufs=2) as xpool, \
                         tc.tile_pool(name="tpool", bufs=3) as tpool, \
                         tc.tile_pool(name="ppsum", bufs=3, space="PSUM") as ppsum, \
                         tc.tile_pool(name="vpsum", bufs=2, space="PSUM") as vpsum:

                        # Q and K projections -> (feature, seq) hi/lo.
                        # Main term fp16; correction terms (lo*hi + hi*lo) in
                        # fp8e4m3 DoubleRow (256-deep contraction, half rate),
                        # accumulated at scale 2^17 and folded in on evac.
                        DRM = mybir.MatmulPerfMode.DoubleRow
                        for (xh_d, x8h_d, x8l_d, wh_d, w8h_d, w8l_d,
                             bias_t, dh, dl) in (
                            (xqh, xq8h, xq8l, wqh, wq8h, wq8l, bq_t, qth, qtl),
                            (xkh, xk8h, xk8l, wkh, wk8h, wk8l, bk_t, kth, ktl),
                        ):
                            wh_t = wpool.tile([P, KC, FPG], F16, tag="w")
                            nc.sync.dma_start(wh_t[:, :KC // 2], drr(wh_d)[:, :KC // 2])
                            nc.sync.dma_start(wh_t[:, KC // 2:], drr(wh_d)[:, KC // 2:])
                            w8h_t = wpool.tile([P, KP, 2, FPG], F8, tag="w8")
                            nc.sync.dma_start(w8h_t[:], drr8(w8h_d))
                            w8l_t = wpool.tile([P, KP, 2, FPG], F8, tag="w8")
                            nc.sync.dma_start(w8l_t[:], drr8(w8l_d))
                            for ip in range(NPAN):
                                isl = slice(ip * IPANEL, (ip + 1) * IPANEL)
                                xh_t = xpool.tile([P, KC, IPANEL], F16, tag="x")
                                nc.sync.dma_start(xh_t[:], drr(xh_d)[:, :, isl])
                                x8h_t = xpool.tile([P, KP, 2, IPANEL], F8, tag="x8")
                                nc.sync.dma_start(x8h_t[:], drr8(x8h_d)[:, :, :, isl])
                                x8l_t = xpool.tile([P, KP, 2, IPANEL], F8, tag="x8")
                                nc.sync.dma_start(x8l_t[:], drr8(x8l_d)[:, :, :, isl])
                                for fc in range(HPG):
                                    fsl = slice(fc * P, (fc + 1) * P)
                                    ps = ppsum.tile([P, IPANEL], F32, tag="pp")
                                    for kc in range(KC):
                                        nc.tensor.matmul(
                                            ps[:], wh_t[:, kc, fsl], xh_t[:, kc, :],
                                            start=(kc == 0), stop=(kc == KC - 1))
                                    psb = ppsum.tile([P, IPANEL], F32, tag="pb")
                                    for kp in range(KP):
                                        nc.tensor.matmul(
                                            psb[:], w8h_t[:, kp, :, fsl],
                                            x8l_t[:, kp, :, :],
                                            start=(kp == 0), stop=False,
                                            perf_mode=DRM)
                                        nc.tensor.matmul(
                                            psb[:], w8l_t[:, kp, :, fsl],
                                            x8h_t[:, kp, :, :],
                                            start=False, stop=(kp == KP - 1),
                                            perf_mode=DRM)
                                    tmp = tpool.tile([P, IPANEL], F32, tag="t")
                                    nc.vector.tensor_scalar(
                                        tmp[:], psb[:], 2.0 ** -17,
                                        bias_t[:, fc:fc + 1],
                                        op0=OP.mult, op1=OP.add)
                                    nc.vector.tensor_tensor(
                                        tmp[:], tmp[:], ps[:], op=OP.add)
                                    nc.gpsimd.tensor_copy(dh[:, fc, isl], tmp[:])
                                    nc.gpsimd.tensor_tensor(
                                        dl[:, fc, isl], tmp[:], dh[:, fc, isl],
                                        op=OP.subtract)

                        # V projection -> natural (seq, feature), single term
                        wv_t = wpool.tile([P, KC, FPG], F16, tag="w")
                        nc.sync.dma_start(wv_t[:], drr(wvh))
                        for ip in range(NPAN):
                            isl = slice(ip * IPANEL, (ip + 1) * IPANEL)
                            xv_t = xpool.tile([P, KC, IPANEL], F16, tag="x")
                            nc.sync.dma_start(xv_t[:], drr(xvh)[:, :, isl])
                            for sub in range(IPANEL // P):
                                ib = ip * (IPANEL // P) + sub
                                ps = vpsum.tile([P, FPG], F32, tag="pv")
                                for kc in range(KC):
                                    nc.tensor.matmul(
                                        ps[:],
                                        xv_t[:, kc, sub * P:(sub + 1) * P],
                                        wv_t[:, kc, :],
                                        start=(kc == 0), stop=(kc == KC - 1))
                                nc.vector.tensor_tensor(
                                    vt[:, ib, :], ps[:], bvb_t[:], op=OP.add)

                    # ---- phase 2: attention; AllGather O^T per head-chunk ----
                    with tc.tile_pool(name="otpool", bufs=1) as otpool, \
                         tc.tile_pool(name="spsum", bufs=5, space="PSUM") as spsum, \
                         tc.tile_pool(name="tpsum", bufs=2, space="PSUM") as tpsum, \
                         tc.tile_pool(name="opsum", bufs=1, space="PSUM") as opsum, \
                         tc.tile_pool(name="ppool", bufs=4) as ppool, \
                         tc.tile_pool(name="ptpool", bufs=6) as ptpool, \
                         tc.tile_pool(name="stats", bufs=6) as stats, \
                         tc.tile_pool(name="dramio", bufs=1, space="DRAM") as dramio:

                        # per-head O^T tiles: head h+1's evacuations carry
                        # no dependency on head h's gather DMA read
                        ots = [otpool.tile([P, L], F16, name=f"ot{hh}")
                               for hh in range(HPG)]

                        def emit_S(h, ib):
                            nj = (ib + 1) * P
                            nch = (nj + 511) // 512
                            isl = slice(ib * P, (ib + 1) * P)
                            mpart = stats.tile([P, 4], F32, tag="mp",
                                               name=f"mp{h}_{ib}")
                            p_sb = ppool.tile([P, L], F16, tag="p",
                                              name=f"p{h}_{ib}")
                            lpart = stats.tile([P, 4], F32, tag="lp",
                                               name=f"lp{h}_{ib}")
                            for jc in range(nch):
                                w = min(512, nj - jc * 512)
                                jsl = slice(jc * 512, jc * 512 + w)
                                diag = jc == nch - 1
                                ps = spsum.tile([P, 512], F32, tag="s",
                                                name=f"sps{h}_{ib}_{jc}")
                                nc.tensor.matmul(
                                    ps[:, :w], qth[:, h, isl], kth[:, h, jsl],
                                    start=True, stop=False)
                                nc.tensor.matmul(
                                    ps[:, :w], qth[:, h, isl], ktl[:, h, jsl],
                                    start=False, stop=False)
                                nc.tensor.matmul(
                                    ps[:, :w], qtl[:, h, isl], kth[:, h, jsl],
                                    start=False, stop=not diag)
                                if diag:
                                    # causal mask on the diagonal 128-block,
                                    # accumulated on the PE: += I.T @ maskh
                                    nc.tensor.matmul(
                                        ps[:, w - P:w], id_t[:], maskh_t[:],
                                        start=False, stop=True)
                                # scores arrive pre-scaled (host folds
                                # sqrt(scale) into Wq/Wk), so the negated
                                # chunk max IS the exp bias: no extra mul
                                nc.vector.reduce_max(
                                    mpart[:, jc:jc + 1], ps[:, :w], axis=AX.X,
                                    negate=True)
                                nc.scalar.activation(
                                    p_sb[:, jsl], ps[:, :w],
                                    ACTF.Exp, bias=mpart[:, jc:jc + 1],
                                    scale=1.0,
                                    accum_out=lpart[:, jc:jc + 1])
                            return p_sb, mpart, lpart

                        def emit_softmax_av(h, ib, p_sb, mpart, lpart):
                            nj = (ib + 1) * P
                            nch = (nj + 511) // 512
                            isl = slice(ib * P, (ib + 1) * P)
                            rmin = stats.tile([P, 1], F32, tag="nm",
                                              name=f"nm{h}_{ib}")
                            nc.vector.tensor_reduce(
                                rmin[:], mpart[:, :nch], axis=AX.X, op=OP.min)
                            # per-chunk correction factors
                            # c = exp(m_jc - m) = exp(rmin - nmpart_jc)
                            cfac = stats.tile([P, 4], F32, tag="cf",
                                              name=f"cf{h}_{ib}")
                            nc.scalar.activation(
                                cfac[:, :nch], mpart[:, :nch],
                                ACTF.Exp, bias=rmin[:], scale=-1.0)
                            lw = stats.tile([P, 4], F32, tag="lw",
                                            name=f"lw{h}_{ib}")
                            nc.vector.tensor_tensor(
                                lw[:, :nch], cfac[:, :nch], lpart[:, :nch],
                                op=OP.mult)
                            lsum = stats.tile([P, 1], F32, tag="ls",
                                              name=f"ls{h}_{ib}")
                            nc.vector.reduce_sum(lsum[:], lw[:, :nch], axis=AX.X)
                            rinv = stats.tile([P, 1], F32, tag="ri",
                                              name=f"ri{h}_{ib}")
                            nc.vector.reciprocal(rinv[:], lsum[:])
                            # P_jc *= c_jc * rinv; transposes batched in
                            # quads into one 512-wide PSUM tile so a single
                            # DVE copy serves 4 blocks, then 4 AV matmuls
                            o_ps = opsum.tile([P, P], F32, tag="o",
                                              name=f"o{h}_{ib}")
                            for jc in range(nch):
                                w = min(512, nj - jc * 512)
                                jsl = slice(jc * 512, jc * 512 + w)
                                nc.vector.tensor_scalar(
                                    p_sb[:, jsl], p_sb[:, jsl],
                                    cfac[:, jc:jc + 1], rinv[:],
                                    op0=OP.mult, op1=OP.mult)
                                jb0 = jc * 4
                                jb1 = min(jc * 4 + 4, ib + 1)
                                nq = jb1 - jb0
                                pt_ps = tpsum.tile([P, 512], F16, tag="pt",
                                                   name=f"pt{h}_{ib}_{jc}")
                                for jb in range(jb0, jb1):
                                    nc.tensor.transpose(
                                        pt_ps[:, (jb - jb0) * P:(jb - jb0 + 1) * P],
                                        p_sb[:, jb * P:(jb + 1) * P],
                                        id_t[:])
                                pt_sb = ptpool.tile([P, 512], F16, tag="ptsb",
                                                    name=f"ptsb{h}_{ib}_{jc}")
                                nc.vector.tensor_copy(
                                    pt_sb[:, :nq * P], pt_ps[:, :nq * P])
                                for jb in range(jb0, jb1):
                                    nc.tensor.matmul(
                                        o_ps[:], vt[:, jb, h * P:(h + 1) * P],
                                        pt_sb[:, (jb - jb0) * P:(jb - jb0 + 1) * P],
                                        start=(jb == 0), stop=(jb == ib))
                            nc.vector.tensor_copy(ots[h][:, isl], o_ps[:])

                        def emit_gather(h):
                            ag_in = dramio.tile([P, L], F16, tag=f"agin{h}",
                                                name=f"agin{h}")
                            nc.sync.dma_start(ag_in[:], ots[h][:])
                            ag_out = dramio.tile([G, P, L], F16, tag=f"agout{h}",
                                                 name=f"agout{h}")
                            if variant in ("nocoll", "x2nc"):
                                for gg in range(G):
                                    nc.sync.dma_start(ag_out[gg], ag_in[:])
                            else:
                                nc.gpsimd.collective_compute(
                                    "AllGather", OP.bypass,
                                    replica_groups=[[0, 1, 2, 3], [4, 5, 6, 7]],
                                    ins=[ag_in[:].opt()], outs=[ag_out[:].opt()])
                            ag_outs.append(ag_out)

                        # 1-unit software pipeline: S(n+1) is emitted before
                        # softmax/AV(n) so the PE always has score matmuls in
                        # program order while unit n waits on ACT/DVE stats.
                        for h in range(HPG):
                            for ib in range(IB):
                                st = emit_S(h, ib)
                                emit_softmax_av(h, ib, *st)
                            emit_gather(h)

                # ---- phase 3: final projection ----
                with tc.tile_pool(name="fpool", bufs=1) as fpool, \
                     tc.tile_pool(name="fopool", bufs=5) as fopool, \
                     tc.tile_pool(name="fpsum", bufs=1, space="PSUM") as fpsum:
                    wo_t = fpool.tile([P, KC, FPG], F16, name=f"wo{_rep}")
                    nc.sync.dma_start(wo_t[:, :KC // 2], drr(woT)[:, :KC // 2])
                    nc.sync.dma_start(wo_t[:, KC // 2:], drr(woT)[:, KC // 2:])
                    at_ts = []
                    for h in range(HPG):
                        at_t = fpool.tile([P, G, L], F16, tag=f"at{h}",
                                          name=f"atld{h}")
                        at_ts.append(at_t)
                    # DMA in consumption order (hc outer, g inner)
                    for h in range(HPG):
                        for g_idx in range(G):
                            nc.sync.dma_start(
                                at_ts[h][:, g_idx, :],
                                ag_outs[h][g_idx].rearrange("p i -> p i"))
                    for half in range(2):
                        ibs = list(range(half * (IB // 2), (half + 1) * (IB // 2)))
                        pss = [fpsum.tile([P, FPG], F32, tag=f"f{i}", name=f"fps{half}_{i}")
                               for i in range(len(ibs))]
                        for hc in range(HPG):
                            for g_idx in range(G):
                                for i, ib in enumerate(ibs):
                                    nc.tensor.matmul(
                                        pss[i][:],
                                        at_ts[hc][:, g_idx, ib * P:(ib + 1) * P],
                                        wo_t[:, g_idx * HPG + hc, :],
                                        start=(hc == 0 and g_idx == 0),
                                        stop=(hc == HPG - 1 and g_idx == G - 1))
                        for i, ib in enumerate(ibs):
                            o_sb = fopool.tile([P, FPG], F32, tag="fo")
                            nc.vector.tensor_tensor(
                                o_sb[:], pss[i][:], bob_t[:], op=OP.add)
                            nc.sync.dma_start(out[ib * P:(ib + 1) * P, :], o_sb[:])

    nc.compile()
    return nc


def _split16(x):
    hi = x.astype(np.float16)
    lo = (x - hi.astype(np.float32)).astype(np.float16)
    return hi, lo


def _prepare_in_maps(q, k, v, Wq, bq, Wk, bk, Wv, bv, Wo, bo):
    mask16 = np.where(
        np.arange(P)[None, :] > np.arange(P)[:, None],
        np.float16(-30000.0), np.float16(0.0)).astype(np.float16)
    ident = np.eye(P, dtype=np.float16)

    f8 = ml_dtypes.float8_e4m3
    xs = {}
    for b in range(B):
        for nm, arr in (("q", q), ("k", k)):
            x = np.ascontiguousarray(arr[b].T, dtype=np.float32)
            hi, lo = _split16(x)
            xs[(nm, b)] = (
                hi,
                hi.astype(np.float32).astype(f8),
                (lo.astype(np.float32) * 2.0 ** 12).astype(f8),
            )
        xs[("v", b)] = np.ascontiguousarray(v[b].T, dtype=np.float32).astype(
            np.float16)

    in_maps = []
    for c in range(8):
        b, g = divmod(c, G)
        F = slice(g * FPG, (g + 1) * FPG)
        rs = np.float32(SCALE ** 0.5)
        wq_h, wq_l = _split16(
            np.ascontiguousarray(Wq[F, :].T, dtype=np.float32) * rs)
        wk_h, wk_l = _split16(
            np.ascontiguousarray(Wk[F, :].T, dtype=np.float32) * rs)
        w8 = {}
        for nm, (wh_, wl_) in (("q", (wq_h, wq_l)), ("k", (wk_h, wk_l))):
            w8[nm] = (
                (wh_.astype(np.float32) * 2.0 ** 5).astype(f8),
                (wl_.astype(np.float32) * 2.0 ** 17).astype(f8),
            )
        in_maps.append({
            "xqh": xs[("q", b)][0],
            "xq8h": xs[("q", b)][1], "xq8l": xs[("q", b)][2],
            "xkh": xs[("k", b)][0],
            "xk8h": xs[("k", b)][1], "xk8l": xs[("k", b)][2],
            "xvh": xs[("v", b)],
            "wqh": wq_h, "wq8h": w8["q"][0], "wq8l": w8["q"][1],
            "wkh": wk_h, "wk8h": w8["k"][0], "wk8l": w8["k"][1],
            "wvh": np.ascontiguousarray(Wv[F, :].T).astype(np.float16),
            "woT": np.ascontiguousarray(Wo[F, :].T).astype(np.float16),
            "bq": np.ascontiguousarray(bq[F]).reshape(FPG, 1).astype(
                np.float32) * rs,
            "bk": np.ascontiguousarray(bk[F]).reshape(FPG, 1).astype(
                np.float32) * rs,
            "bvb": np.broadcast_to(bv[F][None, :], (P, FPG)).astype(np.float32),
            "bob": np.broadcast_to(bo[F][None, :], (P, FPG)).astype(np.float32),
            "maskh": mask16,
            "identd": ident,
        })
    return in_maps


def kernel(**inputs) -> np.ndarray:
    global _COMPILED
    from concourse.bass_utils import run_bass_kernel_spmd

    if _COMPILED is None:
        _COMPILED = _build()
    nc = _COMPILED

    in_maps = _prepare_in_maps(**inputs)
    res = run_bass_kernel_spmd(nc, in_maps, list(range(8)))

    outp = np.empty((B, L, D), dtype=np.float32)
    for c in range(8):
        b, g = divmod(c, G)
        outp[b, :, g * FPG:(g + 1) * FPG] = res.results[c]["out"]
    return outp


if __name__ == "__main__":
    rng = np.random.default_rng(1)
    ins = {
        "q": rng.standard_normal((B, L, D), dtype=np.float32),
        "k": rng.standard_normal((B, L, D), dtype=np.float32),
        "v": rng.standard_normal((B, L, D), dtype=np.float32),
        "Wq": rng.standard_normal((D, D), dtype=np.float32) * 0.02,
        "bq": rng.standard_normal(D).astype(np.float32) * 0.02,
        "Wk": rng.standard_normal((D, D), dtype=np.float32) * 0.02,
        "bk": rng.standard_normal(D).astype(np.float32) * 0.02,
        "Wv": rng.standard_normal((D, D), dtype=np.float32) * 0.02,
        "bv": rng.standard_normal(D).astype(np.float32) * 0.02,
        "Wo": rng.standard_normal((D, D), dtype=np.float32) * 0.02,
        "bo": rng.standard_normal(D).astype(np.float32) * 0.02,
    }
    o = kernel(**ins)
    print("kernel ran, out shape", o.shape)



# revision 47
# speedup vs baseline: 1.0699x; 1.0699x over previous
"""Trainium2 Bass kernel for causal multi-head attention (B=2, L=2048, D=2048,
H=16 heads, DH=128), sharded over 8 NeuronCores.

Sharding: core c handles batch b=c//4 and head-group g=c%4 (4 heads = 512
features). The only cross-core communication is a per-head-chunk fp16
AllGather of attention outputs within each 4-core batch group.

Precision scheme (fp16 datapath, fp32 PSUM accumulation):
- Softmax temperature is sqrt(128) (reference multiplies scores by
  sqrt(d_head)), so score errors are amplified ~11x before exp.
- Q/K projections run compensated (~22 effective bits): fp16 hi*hi main term
  + the two cross terms (lo*hi, hi*lo) in fp8e4m3 DoubleRow, accumulated in a
  second PSUM at scale 2^17 and folded in during evacuation (ACT).
- qh/kh are re-split on device into fp16 hi + e5m2 (hi,lo) planes at natural
  scale; scores S = hi*hi fp16 matmul + ONE fp8e5m2 DoubleRow matmul
  computing both cross terms (qhi*klo + qlo*khi), accumulating directly into
  the same PSUM (no scale fold needed since e5m2 holds natural scale).
- V path, P = softmax(S), attention output, and Wo run in plain fp16.

Layout/overlap notes:
- All x/w DRAM operands are staged host-side in exactly the SBUF tile order
  (partition-major), so every load is 128 contiguous descriptors instead of
  2048 — the DMA issue queue (SP.SEQ) was the v2 bottleneck.
- P^T is produced by ONE DMA-engine transpose per 128-row block
  (SBUF->SBUF), alternating between the SP and ACT hardware DGE queues;
  this replaces PE transposes + PSUM evacuation copies.
- Attention runs a 1-block software pipeline (S of block n+1 is emitted
  before softmax/AV of block n) so the PE always has score matmuls queued
  while the softmax chain (DVE max / ACT exp / Pool scale / DMA transpose)
  drains.
- Wo weights and the gathered activations prefetch during attention; each
  head's gathered activations load in strips interleaved into the next
  head's block loop to avoid HWDGE bursts at head boundaries.
"""
import sys

sys.path.insert(0, "/opt/trn_rl_repo")

import numpy as np
import ml_dtypes

B, L, D, H = 2, 2048, 2048, 16
DH = D // H          # 128
G = 4                # head-groups (tensor-parallel degree per batch)
HPG = H // G         # heads per group = 4
FPG = HPG * DH       # features per group = 512
P = 128
SCALE = float(DH) ** 0.5
KC = D // P          # 16 contraction chunks
IB = L // P          # 16 seq blocks of 128
IPANEL = 512         # projection moving-dim panel
NPAN = L // IPANEL   # 4
KP = D // 256        # 8 DoubleRow contraction chunks

_COMPILED = None


def _build(variant="main"):
    import concourse.bacc as bacc
    import concourse.tile as tile
    from concourse import mybir
    from contextlib import ExitStack

    F32 = mybir.dt.float32
    F16 = mybir.dt.float16
    F8 = mybir.dt.float8e4
    F8E5 = mybir.dt.float8e5
    AX = mybir.AxisListType
    OP = mybir.AluOpType
    ACTF = mybir.ActivationFunctionType
    DRM = mybir.MatmulPerfMode.DoubleRow

    nc = bacc.Bacc("TRN2", target_bir_lowering=False, debug=False, num_devices=8)

    # ---- DRAM I/O (all x/w pre-arranged host-side in tile order) ----
    xqh = nc.dram_tensor("xqh", [P, NPAN, KC, IPANEL], F16, kind="ExternalInput")
    xkh = nc.dram_tensor("xkh", [P, NPAN, KC, IPANEL], F16, kind="ExternalInput")
    xvh = nc.dram_tensor("xvh", [P, NPAN, KC, IPANEL], F16, kind="ExternalInput")
    wqh = nc.dram_tensor("wqh", [P, KC, FPG], F16, kind="ExternalInput")
    wkh = nc.dram_tensor("wkh", [P, KC, FPG], F16, kind="ExternalInput")
    wvh = nc.dram_tensor("wvh", [P, KC, FPG], F16, kind="ExternalInput")
    xq8h = nc.dram_tensor("xq8h", [P, NPAN, KP, 2, IPANEL], F8, kind="ExternalInput")
    xq8l = nc.dram_tensor("xq8l", [P, NPAN, KP, 2, IPANEL], F8, kind="ExternalInput")
    xk8h = nc.dram_tensor("xk8h", [P, NPAN, KP, 2, IPANEL], F8, kind="ExternalInput")
    xk8l = nc.dram_tensor("xk8l", [P, NPAN, KP, 2, IPANEL], F8, kind="ExternalInput")
    wq8h = nc.dram_tensor("wq8h", [P, KP, 2, FPG], F8, kind="ExternalInput")
    wq8l = nc.dram_tensor("wq8l", [P, KP, 2, FPG], F8, kind="ExternalInput")
    wk8h = nc.dram_tensor("wk8h", [P, KP, 2, FPG], F8, kind="ExternalInput")
    wk8l = nc.dram_tensor("wk8l", [P, KP, 2, FPG], F8, kind="ExternalInput")
    woT = nc.dram_tensor("woT", [P, KC, FPG], F16, kind="ExternalInput")
    bq = nc.dram_tensor("bq", [P, HPG], F32, kind="ExternalInput")
    bk = nc.dram_tensor("bk", [P, HPG], F32, kind="ExternalInput")
    bvb = nc.dram_tensor("bvb", [P, FPG], F32, kind="ExternalInput")
    bob = nc.dram_tensor("bob", [P, FPG], F32, kind="ExternalInput")
    maskh = nc.dram_tensor("maskh", [P, P], F16, kind="ExternalInput")
    identd = nc.dram_tensor("identd", [P, P], F16, kind="ExternalInput")
    out = nc.dram_tensor("out", [L, FPG], F32, kind="ExternalOutput")
    if variant == "timing":
        chain = nc.dram_tensor("chain", [1, 8], F32, kind="ExternalInput")
        dummy = nc.dram_tensor("chaino", [1, 8], F32, kind="ExternalOutput")

    with tile.TileContext(nc) as tc:
        with ExitStack() as ctx:
            consts = ctx.enter_context(tc.tile_pool(name="consts", bufs=1))

            maskh_t = consts.tile([P, P], F16)
            nc.scalar.dma_start(maskh_t[:], maskh[:])
            id_t = consts.tile([P, P], F16)
            nc.scalar.dma_start(id_t[:], identd[:])
            bq_t = consts.tile([P, HPG], F32)
            nc.sync.dma_start(bq_t[:], bq[:])
            bk_t = consts.tile([P, HPG], F32)
            nc.sync.dma_start(bk_t[:], bk[:])
            bvb_t = consts.tile([P, FPG], F32)
            bob_t = consts.tile([P, FPG], F32)
            if variant == "timing":
                ch_t = consts.tile([1, 8], F32)
                nc.sync.dma_start(ch_t[:], chain[:])
                nc.sync.dma_start(dummy[:], ch_t[:])

            NREP = {"x4": 4, "x2": 2, "x2nc": 2}.get(variant, 1)
            for _rep in range(NREP):
                ag_outs = []
                with tc.tile_pool(name="qkv", bufs=1) as qkv:
                    qth = qkv.tile([P, HPG, L], F16)       # (d, head, seq) hi
                    kth = qkv.tile([P, HPG, L], F16)
                    q8t = qkv.tile([P, HPG, 2, L], F8E5)   # planes: 0=hi 1=lo
                    k8t = qkv.tile([P, HPG, 2, L], F8E5)   # planes: 0=lo 1=hi
                    vt = qkv.tile([P, IB, FPG], F16)       # (seq%128, blk, feat)

                    # ---- phase 1: projections ----
                    with tc.tile_pool(name="wpool", bufs=2) as wpool, \
                         tc.tile_pool(name="xpool", bufs=2) as xpool, \
                         tc.tile_pool(name="tpool", bufs=2) as tpool, \
                         tc.tile_pool(name="ppsum", bufs=3, space="PSUM") as ppsum, \
                         tc.tile_pool(name="vpsum", bufs=2, space="PSUM") as vpsum:

                        # Q and K projections -> (feature, seq) fp16 hi +
                        # e5m2 (hi, lo) planes. Main term fp16; corrections
                        # (lo*hi + hi*lo) in fp8e4m3 DoubleRow at scale 2^17.
                        def proj_qk(xh_d, x8h_d, x8l_d, wh_d, w8h_d, w8l_d,
                                    bias_t, dh, d8, lo_first, first=False):
                            wh_t = wpool.tile([P, KC, FPG], F16, tag="w")
                            nc.sync.dma_start(wh_t[:, :KC // 2], wh_d[:, :KC // 2])
                            w8h_t = wpool.tile([P, KP, 2, FPG], F8, tag="w8")
                            w8l_t = wpool.tile([P, KP, 2, FPG], F8, tag="w8")
                            for ip in range(NPAN):
                                isl = slice(ip * IPANEL, (ip + 1) * IPANEL)
                                xh_t = xpool.tile([P, KC, IPANEL], F16, tag="x")
                                nc.sync.dma_start(xh_t[:], xh_d[:, ip])
                                if ip == 0:
                                    nc.sync.dma_start(wh_t[:, KC // 2:],
                                                      wh_d[:, KC // 2:])
                                    nc.sync.dma_start(w8h_t[:], w8h_d[:])
                                    nc.sync.dma_start(w8l_t[:], w8l_d[:])
                                x8h_t = xpool.tile([P, KP, 2, IPANEL], F8, tag="x8h")
                                nc.sync.dma_start(x8h_t[:], x8h_d[:, ip])
                                x8l_t = xpool.tile([P, KP, 2, IPANEL], F8, tag="x8l")
                                nc.sync.dma_start(x8l_t[:], x8l_d[:, ip])
                                for fc in range(HPG):
                                    fsl = slice(fc * P, (fc + 1) * P)
                                    ps = ppsum.tile([P, IPANEL], F32, tag="pp")
                                    for kc in range(KC):
                                        nc.tensor.matmul(
                                            ps[:], wh_t[:, kc, fsl], xh_t[:, kc, :],
                                            start=(kc == 0), stop=(kc == KC - 1))
                                    psb = ppsum.tile([P, IPANEL], F32, tag="pb")
                                    for kp in range(KP):
                                        nc.tensor.matmul(
                                            psb[:], w8h_t[:, kp, :, fsl],
                                            x8l_t[:, kp, :, :],
                                            start=(kp == 0), stop=False,
                                            perf_mode=DRM)
                                        nc.tensor.matmul(
                                            psb[:], w8l_t[:, kp, :, fsl],
                                            x8h_t[:, kp, :, :],
                                            start=False, stop=(kp == KP - 1),
                                            perf_mode=DRM)
                                    # evacuation: ACT folds corr, Pool adds
                                    # main + writes fp16 hi, DVE writes the
                                    # e5m2 planes (hi plane index depends on
                                    # DR pairing: q planes (hi,lo), k (lo,hi))
                                    tmp = tpool.tile([P, IPANEL], F32, tag="t")
                                    nc.scalar.activation(
                                        tmp[:], psb[:], ACTF.Identity,
                                        bias=bias_t[:, fc:fc + 1],
                                        scale=2.0 ** -17)
                                    tmp2 = tpool.tile([P, IPANEL], F32, tag="t2")
                                    nc.vector.tensor_tensor(
                                        tmp2[:], tmp[:], ps[:], op=OP.add)
                                    nc.gpsimd.tensor_copy(dh[:, fc, isl], tmp2[:])
                                    hi_pl, lo_pl = (1, 0) if lo_first else (0, 1)
                                    nc.vector.tensor_copy(
                                        d8[:, fc, hi_pl, isl], tmp2[:])
                                    nc.vector.tensor_tensor(
                                        d8[:, fc, lo_pl, isl], tmp2[:],
                                        dh[:, fc, isl], op=OP.subtract)

                        # Order V, Q, K: V needs the least data (fastest
                        # start), K's outputs are the freshest when attention
                        # starts, and V's PSUM banks (reused by the attention
                        # score pool) are long released by then.
                        wv_t = wpool.tile([P, KC, FPG], F16, tag="w")
                        for ip in range(NPAN):
                            xv_t = xpool.tile([P, KC, IPANEL], F16, tag="x")
                            if ip == 0:
                                # interleave weight/x quarters so the first
                                # matmul's operands land first
                                for xc in range(4):
                                    xsl = slice(xc * KC // 4,
                                                (xc + 1) * KC // 4)
                                    nc.sync.dma_start(xv_t[:, xsl],
                                                      xvh[:, ip, xsl])
                                    nc.sync.dma_start(wv_t[:, xsl],
                                                      wvh[:, xsl])
                            else:
                                nc.sync.dma_start(xv_t[:], xvh[:, ip])
                            if ip == 1 and _rep == 0:
                                nc.scalar.dma_start(bvb_t[:], bvb[:])
                                nc.scalar.dma_start(bob_t[:], bob[:])
                            for sub in range(IPANEL // P):
                                ib = ip * (IPANEL // P) + sub
                                ps = vpsum.tile([P, FPG], F32, tag="pv")
                                for kc in range(KC):
                                    nc.tensor.matmul(
                                        ps[:],
                                        xv_t[:, kc, sub * P:(sub + 1) * P],
                                        wv_t[:, kc, :],
                                        start=(kc == 0), stop=(kc == KC - 1))
                                nc.vector.tensor_tensor(
                                    vt[:, ib, :], ps[:], bvb_t[:], op=OP.add)

                        proj_qk(xqh, xq8h, xq8l, wqh, wq8h, wq8l, bq_t,
                                qth, q8t, False)
                        proj_qk(xkh, xk8h, xk8l, wkh, wk8h, wk8l, bk_t,
                                kth, k8t, True)

                    # ---- phase 2+3 tiles: Wo weights prefetched during
                    # attention; gathered activations stream in phase 3 ----
                    with tc.tile_pool(name="ph3", bufs=1) as ph3:
                        wo_t = ph3.tile([P, KC, FPG], F16, name=f"wo{_rep}")
                        nc.scalar.dma_start(wo_t[:, :KC // 2], woT[:, :KC // 2])
                        nc.scalar.dma_start(wo_t[:, KC // 2:], woT[:, KC // 2:])

                        with tc.tile_pool(name="otpool", bufs=1) as otpool, \
                             tc.tile_pool(name="spsum", bufs=6, space="PSUM") as spsum, \
                             tc.tile_pool(name="opsum", bufs=2, space="PSUM") as opsum, \
                             tc.tile_pool(name="ppool", bufs=6) as ppool, \
                             tc.tile_pool(name="ptpool", bufs=5) as ptpool, \
                             tc.tile_pool(name="stats", bufs=8) as stats, \
                             tc.tile_pool(name="dramio", bufs=1, space="DRAM") as dramio:

                            ots = [otpool.tile([P, L], F16, name=f"ot{hh}")
                                   for hh in range(HPG)]
                            # pending gathered-activation strip loads,
                            # drained into the next head's block loop
                            at_pending = []

                            def emit_S(h, ib):
                                nj = (ib + 1) * P
                                nch = (nj + 511) // 512
                                isl = slice(ib * P, (ib + 1) * P)
                                mpart = stats.tile([P, 4], F32, tag="mp",
                                                   name=f"mp{h}_{ib}")
                                p_sb = ppool.tile([P, L], F16, tag="p",
                                                  name=f"p{h}_{ib}")
                                lpart = stats.tile([P, 4], F32, tag="lp",
                                                   name=f"lp{h}_{ib}")
                                chunks = []
                                for jc in range(nch):
                                    w = min(512, nj - jc * 512)
                                    jsl = slice(jc * 512, jc * 512 + w)
                                    diag = jc == nch - 1
                                    ps = spsum.tile([P, 512], F32, tag="s",
                                                    name=f"sps{h}_{ib}_{jc}")
                                    nc.tensor.matmul(
                                        ps[:, :w], qth[:, h, isl], kth[:, h, jsl],
                                        start=True, stop=False)
                                    nc.tensor.matmul(
                                        ps[:, :w], q8t[:, h, :, isl],
                                        k8t[:, h, :, jsl],
                                        start=False, stop=not diag,
                                        perf_mode=DRM)
                                    if diag:
                                        # causal mask on the diagonal block,
                                        # accumulated on the PE: += I.T @ mask
                                        nc.tensor.matmul(
                                            ps[:, w - P:w], id_t[:], maskh_t[:],
                                            start=False, stop=True)
                                    chunks.append((ps, w, jsl, jc))
                                return p_sb, mpart, lpart, chunks

                            def emit_S_stats(h, ib, p_sb, mpart, lpart,
                                             chunks):
                                # scores arrive pre-scaled (host folds
                                # sqrt(scale) into Wq/Wk): negated chunk
                                # max IS the exp bias
                                for ps, w, jsl, jc in chunks:
                                    nc.vector.reduce_max(
                                        mpart[:, jc:jc + 1], ps[:, :w],
                                        axis=AX.X, negate=True)
                                    nc.scalar.activation(
                                        p_sb[:, jsl], ps[:, :w],
                                        ACTF.Exp, bias=mpart[:, jc:jc + 1],
                                        scale=1.0,
                                        accum_out=lpart[:, jc:jc + 1])

                            def emit_softmax_av(h, ib, p_sb, mpart, lpart,
                                                chunks):
                                nj = (ib + 1) * P
                                nch = (nj + 511) // 512
                                isl = slice(ib * P, (ib + 1) * P)
                                rmin = stats.tile([P, 1], F32, tag="nm",
                                                  name=f"nm{h}_{ib}")
                                nc.vector.tensor_reduce(
                                    rmin[:], mpart[:, :nch], axis=AX.X, op=OP.min)
                                # per-chunk correction c = exp(m_jc - m)
                                cfac = stats.tile([P, 4], F32, tag="cf",
                                                  name=f"cf{h}_{ib}")
                                nc.scalar.activation(
                                    cfac[:, :nch], mpart[:, :nch],
                                    ACTF.Exp, bias=rmin[:], scale=-1.0)
                                lw = stats.tile([P, 4], F32, tag="lw",
                                                name=f"lw{h}_{ib}")
                                nc.vector.tensor_tensor(
                                    lw[:, :nch], cfac[:, :nch], lpart[:, :nch],
                                    op=OP.mult)
                                lsum = stats.tile([P, 1], F32, tag="ls",
                                                  name=f"ls{h}_{ib}")
                                nc.vector.reduce_sum(lsum[:], lw[:, :nch],
                                                     axis=AX.X)
                                rinv = stats.tile([P, 1], F32, tag="ri",
                                                  name=f"ri{h}_{ib}")
                                nc.vector.reciprocal(rinv[:], lsum[:])
                                # P_jc *= c_jc * rinv (DVE 4x); transposes
                                # in halves so first-half AV matmuls overlap
                                # the second half's transpose latency
                                for jc in range(nch):
                                    w = min(512, nj - jc * 512)
                                    jsl = slice(jc * 512, jc * 512 + w)
                                    nc.vector.tensor_scalar(
                                        p_sb[:, jsl], p_sb[:, jsl],
                                        cfac[:, jc:jc + 1], rinv[:],
                                        op0=OP.mult, op1=OP.mult)
                                pt_sb = ptpool.tile([P, IB, P], F16, tag="ptsb",
                                                    name=f"ptsb{h}_{ib}")
                                o_tile = opsum.tile([P, P], F32, tag="o",
                                                    name=f"o{h}_{ib}")
                                o_ps = o_tile[:]
                                hsplit = [(0, nch)] if nch <= 2 else \
                                    [(0, 2), (2, nch)]
                                for hp, (c0, c1) in enumerate(hsplit):
                                    j0 = c0 * 4
                                    j1 = min(c1 * 4, ib + 1)
                                    nc.sync.dma_start_transpose(
                                        pt_sb[:, j0:j1, :],
                                        p_sb[:, j0 * P:j1 * P])
                                for jb in range(ib + 1):
                                    nc.tensor.matmul(
                                        o_ps, vt[:, jb, h * P:(h + 1) * P],
                                        pt_sb[:, jb, :],
                                        start=(jb == 0), stop=(jb == ib))
                                nc.vector.tensor_copy(ots[h][:, isl], o_ps)

                            def emit_gather(h):
                                if h == HPG - 1 and h3half and \
                                        variant in ("nocoll", "x2nc"):
                                    ag_in = h3half["ag_in"]
                                    ag_out = h3half["ag_out"]
                                    nc.scalar.dma_start(
                                        ag_in[:, L // 2:], ots[h][:, L // 2:])
                                    for gg in range(G):
                                        def cp2(gg=gg):
                                            nc.scalar.dma_start(
                                                ag_out[gg][:, L // 2:],
                                                ag_in[:, L // 2:])
                                        at_pending.append(cp2)
                                    ag_outs.append(ag_out)
                                    return
                                ag_in = dramio.tile([P, L], F16, tag=f"agin{h}",
                                                    name=f"agin{h}")
                                nc.scalar.dma_start(ag_in[:], ots[h][:])
                                ag_out = dramio.tile([G, P, L], F16,
                                                     tag=f"agout{h}",
                                                     name=f"agout{h}")
                                if variant in ("nocoll", "x2nc"):
                                    # strip the gather-emulation copies so
                                    # they don't burst ahead of the next
                                    # head's P^T transposes at the DMA engines
                                    for gg in range(G):
                                        def cp(gg=gg, ag_out=ag_out,
                                               ag_in=ag_in):
                                            nc.scalar.dma_start(
                                                ag_out[gg], ag_in[:])
                                        at_pending.append(cp)
                                else:
                                    nc.gpsimd.collective_compute(
                                        "AllGather", OP.bypass,
                                        replica_groups=[[0, 1, 2, 3], [4, 5, 6, 7]],
                                        ins=[ag_in[:].opt()], outs=[ag_out[:].opt()])
                                ag_outs.append(ag_out)

                            def drain_at(n):
                                for _ in range(n):
                                    if not at_pending:
                                        return
                                    at_pending.pop(0)()

                            # 2-block software pipeline (crossing head
                            # boundaries): S of blocks n+1, n+2 are emitted
                            # before softmax/AV of block n so the PE always
                            # has score matmuls queued while the softmax
                            # chain (DVE/ACT/DMA transpose) drains
                            # 3-stage pipeline: S matmuls of block n,
                            # stats (max+exp) of block n-1, softmax/AV of
                            # block n-2 — keeps each softmax's pscales at
                            # the DVE FIFO head (never behind future maxes)
                            pend = []

                            h3half = {}

                            def drain_one():
                                e = pend.pop(0)
                                emit_softmax_av(*e)
                                if e[1] % 2 == 0:
                                    drain_at(1)
                                if e[0] == HPG - 1 and e[1] == IB // 2 - 1:
                                    # last head: gather the finished first
                                    # half mid-head so phase 3's first
                                    # activation loads start early
                                    ag_in = dramio.tile(
                                        [P, L], F16, tag="aginL", name="aginL")
                                    ag_out = dramio.tile(
                                        [G, P, L], F16, tag="agoutL",
                                        name="agoutL")
                                    h3half.update(ag_in=ag_in, ag_out=ag_out)
                                    if variant in ("nocoll", "x2nc"):
                                        nc.scalar.dma_start(
                                            ag_in[:, :L // 2],
                                            ots[e[0]][:, :L // 2])
                                        for gg in range(G):
                                            def cp(gg=gg, ag_out=ag_out,
                                                   ag_in=ag_in):
                                                nc.scalar.dma_start(
                                                    ag_out[gg][:, :L // 2],
                                                    ag_in[:, :L // 2])
                                            at_pending.append(cp)
                                if e[1] == IB - 1:
                                    emit_gather(e[0])

                            def nch_of(e):
                                return (e[1] * P + P + 511) // 512

                            for h in range(HPG):
                                for ib in range(IB):
                                    pend.append((h, ib, *emit_S(h, ib)))
                                    if len(pend) >= 2:
                                        emit_S_stats(*pend[-2])
                                    # adaptive depth: drain when the pending
                                    # blocks' score chunks would exceed the
                                    # spsum PSUM banks (deep pipeline for
                                    # small early blocks, shallow for late)
                                    while (sum(nch_of(e) for e in pend) > 6
                                           or len(pend) > 5):
                                        drain_one()
                            emit_S_stats(*pend[-1])
                            while pend:
                                drain_one()
                            drain_at(len(at_pending))

                        # ---- phase 3: final projection, streamed in
                        # quarters (gathered activations load per quarter,
                        # double buffered, overlapping the Wo matmuls) ----
                        with tc.tile_pool(name="fapool", bufs=2) as fapool, \
                             tc.tile_pool(name="fopool", bufs=5) as fopool, \
                             tc.tile_pool(name="fpsum", bufs=2, space="PSUM") as fpsum:
                            NQ = 4
                            QW = L // NQ   # 512 seq cols per quarter
                            for q in range(NQ):
                                qsl = slice(q * QW, (q + 1) * QW)
                                atq = fapool.tile([P, HPG, G, QW], F16,
                                                  tag="atq", name=f"atq{q}")
                                for hc in range(HPG):
                                    nc.scalar.dma_start(
                                        atq[:, hc],
                                        ag_outs[hc].rearrange(
                                            "g p l -> p g l")[:, :, qsl])
                                ibs = list(range(q * (IB // NQ),
                                                 (q + 1) * (IB // NQ)))
                                pss = [fpsum.tile([P, FPG], F32, tag=f"f{i}",
                                                  name=f"fps{q}_{i}")
                                       for i in range(len(ibs))]
                                for hc in range(HPG):
                                    for g_idx in range(G):
                                        for i, ib in enumerate(ibs):
                                            nc.tensor.matmul(
                                                pss[i][:],
                                                atq[:, hc, g_idx,
                                                    i * P:(i + 1) * P],
                                                wo_t[:, g_idx * HPG + hc, :],
                                                start=(hc == 0 and g_idx == 0),
                                                stop=(hc == HPG - 1
                                                      and g_idx == G - 1))
                                for i, ib in enumerate(ibs):
                                    o_sb = fopool.tile([P, FPG], F32, tag="fo")
                                    nc.vector.tensor_tensor(
                                        o_sb[:], pss[i][:], bob_t[:], op=OP.add)
                                    nc.sync.dma_start(
                                        out[ib * P:(ib + 1) * P, :], o_sb[:])

    nc.compile()
    return nc


def _split16(x):
    hi = x.astype(np.float16)
    lo = (x - hi.astype(np.float32)).astype(np.float16)
    return hi, lo


def _tile16(x):
    # [D, L] -> [P, NPAN, KC, IPANEL]:  (kc*128+p, ip*512+c) -> [p, ip, kc, c]
    return np.ascontiguousarray(
        x.reshape(KC, P, NPAN, IPANEL).transpose(1, 2, 0, 3))


def _tile8(x):
    # [D, L] -> [P, NPAN, KP, 2, IPANEL]: (kp*256+r*128+p, ip*512+c)
    return np.ascontiguousarray(
        x.reshape(KP, 2, P, NPAN, IPANEL).transpose(2, 3, 0, 1, 4))


def _tilew(w):
    # [D, FPG] -> [P, KC, FPG]
    return np.ascontiguousarray(w.reshape(KC, P, FPG).transpose(1, 0, 2))


def _tilew8(w):
    # [D, FPG] -> [P, KP, 2, FPG]
    return np.ascontiguousarray(w.reshape(KP, 2, P, FPG).transpose(2, 0, 1, 3))


def _prepare_in_maps(q, k, v, Wq, bq, Wk, bk, Wv, bv, Wo, bo):
    mask16 = np.where(
        np.arange(P)[None, :] > np.arange(P)[:, None],
        np.float16(-30000.0), np.float16(0.0)).astype(np.float16)
    ident = np.eye(P, dtype=np.float16)

    f8 = ml_dtypes.float8_e4m3
    xs = {}
    for b in range(B):
        for nm, arr in (("q", q), ("k", k)):
            x = np.ascontiguousarray(arr[b].T, dtype=np.float32)
            hi, lo = _split16(x)
            xs[(nm, b)] = (
                _tile16(hi),
                _tile8(hi.astype(np.float32).astype(f8)),
                _tile8((lo.astype(np.float32) * 2.0 ** 12).astype(f8)),
            )
        xs[("v", b)] = _tile16(
            np.ascontiguousarray(v[b].T, dtype=np.float32).astype(np.float16))

    in_maps = []
    for c in range(8):
        b, g = divmod(c, G)
        F = slice(g * FPG, (g + 1) * FPG)
        rs = np.float32(SCALE ** 0.5)
        wq_h, wq_l = _split16(
            np.ascontiguousarray(Wq[F, :].T, dtype=np.float32) * rs)
        wk_h, wk_l = _split16(
            np.ascontiguousarray(Wk[F, :].T, dtype=np.float32) * rs)
        w8 = {}
        for nm, (wh_, wl_) in (("q", (wq_h, wq_l)), ("k", (wk_h, wk_l))):
            w8[nm] = (
                _tilew8((wh_.astype(np.float32) * 2.0 ** 5).astype(f8)),
                _tilew8((wl_.astype(np.float32) * 2.0 ** 17).astype(f8)),
            )
        in_maps.append({
            "xqh": xs[("q", b)][0],
            "xq8h": xs[("q", b)][1], "xq8l": xs[("q", b)][2],
            "xkh": xs[("k", b)][0],
            "xk8h": xs[("k", b)][1], "xk8l": xs[("k", b)][2],
            "xvh": xs[("v", b)],
            "wqh": _tilew(wq_h), "wq8h": w8["q"][0], "wq8l": w8["q"][1],
            "wkh": _tilew(wk_h), "wk8h": w8["k"][0], "wk8l": w8["k"][1],
            "wvh": _tilew(np.ascontiguousarray(Wv[F, :].T).astype(np.float16)),
            "woT": _tilew(np.ascontiguousarray(Wo[F, :].T).astype(np.float16)),
            "bq": np.ascontiguousarray(
                (bq[F] * rs).astype(np.float32).reshape(HPG, P).T),
            "bk": np.ascontiguousarray(
                (bk[F] * rs).astype(np.float32).reshape(HPG, P).T),
            "bvb": np.broadcast_to(bv[F][None, :], (P, FPG)).astype(np.float32),
            "bob": np.broadcast_to(bo[F][None, :], (P, FPG)).astype(np.float32),
            "maskh": mask16,
            "identd": ident,
        })
    return in_maps


def kernel(**inputs) -> np.ndarray:
    global _COMPILED
    from concourse.bass_utils import run_bass_kernel_spmd

    if _COMPILED is None:
        _COMPILED = _build()
    nc = _COMPILED

    in_maps = _prepare_in_maps(**inputs)
    res = run_bass_kernel_spmd(nc, in_maps, list(range(8)))

    outp = np.empty((B, L, D), dtype=np.float32)
    for c in range(8):
        b, g = divmod(c, G)
        outp[b, :, g * FPG:(g + 1) * FPG] = res.results[c]["out"]
    return outp


if __name__ == "__main__":
    rng = np.random.default_rng(1)
    ins = {
        "q": rng.standard_normal((B, L, D), dtype=np.float32),
        "k": rng.standard_normal((B, L, D), dtype=np.float32),
        "v": rng.standard_normal((B, L, D), dtype=np.float32),
        "Wq": rng.standard_normal((D, D), dtype=np.float32) * 0.02,
        "bq": rng.standard_normal(D).astype(np.float32) * 0.02,
        "Wk": rng.standard_normal((D, D), dtype=np.float32) * 0.02,
        "bk": rng.standard_normal(D).astype(np.float32) * 0.02,
        "Wv": rng.standard_normal((D, D), dtype=np.float32) * 0.02,
        "bv": rng.standard_normal(D).astype(np.float32) * 0.02,
        "Wo": rng.standard_normal((D, D), dtype=np.float32) * 0.02,
        "bo": rng.standard_normal(D).astype(np.float32) * 0.02,
    }
    o = kernel(**ins)
    print("kernel ran, out shape", o.shape)
